# revision 61
# baseline (speedup 1.0000x reference)
"""3-layer GAT + graph pooling + MLP on 8 Trainium2 NeuronCores (Bass).

fp16 edge pipeline, non-redundant dense, no per-edge s_dst gather.

Sharding: core c owns dst-nodes [c*NLOC, (c+1)*NLOC) and their in-edges.
Per layer:
  dense:  OWN nodes only: htab rows [h(192) | s_src(4) | s_dst(4) | pad]
          as 256-elem fp16 rows (512B, the dma_gather minimum granularity
          with latency-multiplier 1) via one augmented-W matmul per tile;
          one AllGather distributes the full table (links ~150GB/s).
  edge:   per dst-tile, BOTH src-regions gathered into one fused block
          (contiguous edge order) so every vector op runs once per tile:
          dma_gather 512B fp16 src rows (SWDGE ring enlarged to 3072
          descs so desc-gen never stalls on transfer drain; regions
          alternate the two SWDGE queues); per-edge s_dst broadcast on
          TensorE: lhsT=ohT (host-precomputed transposed one-hot, fp16)
          x rhs=own s_dst rows -> [128e, H] psum, no second gather;
          w = exp(lrelu(s_src + s_dst) - 4) in f32 (the -4 shift cancels
          in the softmax and keeps w in fp16-safe range downstream);
          sc = h * w_bcast in one DVE op; aggregation
          psum += oh^T @ [w*h | w] with oh built in ONE DVE is_equal per
          tile (iota_rep vs partition-major dstv, fp16 exact for 0..127).
  post:   1/den, +bias, ELU in f32 (max-pool path needs f32 accuracy),
          fp16 copy transposed to channel-major x_own (SBUF-resident)
          for the next layer's dense.
Regions: htab rows core-major; region r = rows [r*25088, (r+1)*25088)
so int16 gather indices fit. Sequential DMAs issue from the sync (SP)
engine queue (HWDGE) to keep the Pool engine free for gather desc-gen.
Pooling: unique-index scatter (+1000 offset) into graph-aligned slots,
one-hot sum-pool on TensorE, 64-wide window max + log-depth same-graph
combine, one small AllGather, MLP computed redundantly. dma_scatter_add
is collision-safe only with unique indices, which these are.
All data-dependent structure (indices, one-hots) is INPUT DATA; the
program is static and identical across cores (SPMD).
"""

import sys
import numpy as np

sys.path.insert(0, "/opt/trn_rl_repo")

H, C = 4, 48
HC = H * C          # 192
NEG = 0.2
BIGNEG = -2.0e30
MAXOFF = 1000.0     # max-pool offset: x3 = elu(...) >= -1, so x3+1000 > 0
ROWB = 256          # htab row elems (bf16): h 192 | s_src 4 | s_dst 4 | pad


def make_cfg(N=50000, E=800000, G=64, NC=8, FEAT=128, WIN=64):
    NLOC = N // NC
    assert NLOC * NC == N
    NSL = ((NLOC + 127) // 128) * 128
    GSLOTS = NC * NSL
    REG = GSLOTS // 2
    assert REG <= 32768
    return dict(N=N, E=E, G=G, NC=NC, FEAT=FEAT, NLOC=NLOC, NSL=NSL,
                GSLOTS=GSLOTS, REG=REG, WIN=WIN)


def _wrap_idx(idx):
    """SWDGE idx layout: element i -> [i % 16, i // 16], replicated to 128
    partitions (one copy per Q7 core)."""
    T = idx.shape[0]
    out = np.ascontiguousarray(idx.reshape(T // 16, 16).T).astype(np.int16)
    return np.tile(out, (8, 1))


def host_prep(cfg, adj, batch):
    N, G, NC = cfg["N"], cfg["G"], cfg["NC"]
    NLOC, NSL, WIN, REG = cfg["NLOC"], cfg["NSL"], cfg["WIN"], cfg["REG"]
    NTL = NSL // 128
    nreg = 2
    src = np.asarray(adj[0], dtype=np.int64)
    dst = np.asarray(adj[1], dtype=np.int64)
    batch = np.asarray(batch, dtype=np.int64)
    src_slot = (src // NLOC) * NSL + (src % NLOC)
    counts_g = np.bincount(batch, minlength=G)

    # ---- edge grouping: (dst-tile, src-region) blocks, each padded %128
    blocks_all = []
    sizes = np.zeros((NC, NTL, nreg), dtype=np.int64)
    for c in range(NC):
        lo = c * NLOC
        esel = np.nonzero((dst >= lo) & (dst < lo + NLOC))[0]
        # sort by src for gather locality
        esel = esel[np.argsort(src_slot[esel], kind="stable")]
        dt_of = (dst[esel] - lo) // 128
        rg_of = src_slot[esel] // REG
        d = {}
        for t in range(NTL):
            for r in range(nreg):
                ee = esel[(dt_of == t) & (rg_of == r)]
                d[(t, r)] = ee
                sizes[c, t, r] = len(ee)
        blocks_all.append(d)
    bsz = np.zeros((NTL, nreg), dtype=np.int64)
    for t in range(NTL):
        for r in range(nreg):
            m = int(sizes[:, t, r].max())
            if r == 0:
                m = max(m, 1)          # ensure >=1 tile so psum gets reset
            bsz[t, r] = -(-m // 128) * 128 if m else 0
    offs = np.zeros((NTL, nreg), dtype=np.int64)
    o = 0
    for t in range(NTL):
        for r in range(nreg):
            offs[t, r] = o
            o += bsz[t, r]
    TOT = int(o)

    # ---- layer-3 graph-aligned slots (for pooling) — as v1
    pad3_meta, pad3_tot = [], 0
    for c in range(NC):
        lo = c * NLOC
        b = batch[lo:lo + NLOC]
        gids, starts = np.unique(b, return_index=True)
        osort = np.argsort(starts)
        gids, starts = gids[osort], starts[osort]
        ends = np.append(starts[1:], NLOC)
        slots = np.empty(NLOC, dtype=np.int64)
        wg, fwin = [], []
        pos = 0
        for g, s, e in zip(gids, starts, ends):
            cnt = e - s
            slots[s:e] = pos + np.arange(cnt)
            nw = -(-cnt // WIN)
            wg += [int(g)] * nw
            fwin += [1] + [0] * (nw - 1)
            pos += nw * WIN
        pad3_meta.append((slots, wg, fwin))
        pad3_tot = max(pad3_tot, pos)
    PAD3 = -(-pad3_tot // 128) * 128
    NW, NT3 = PAD3 // WIN, PAD3 // 128
    assert NW <= 128
    SMAX = int(bsz.max()) // 128
    SMAXT = int(bsz.sum(axis=1).max()) // 128
    cfg.update(TOT=TOT, bsz=bsz, offs=offs, PAD3=PAD3, NW=NW, NT3=NT3,
               nreg=nreg, NTL=NTL, SMAX=SMAX, SMAXT=SMAXT)

    data = []
    strides = [1, 2, 4, 8, 16, 32]
    for c in range(NC):
        lo = c * NLOC
        slots3, wg, fwin = pad3_meta[c]
        g1 = np.zeros(TOT, dtype=np.int64)
        dstv = np.full(TOT, 999.0, dtype=np.float32)
        for t in range(NTL):
            for r in range(nreg):
                ee = blocks_all[c][(t, r)]
                i0 = int(offs[t, r])
                g1[i0:i0 + len(ee)] = src_slot[ee] - r * REG
                dstv[i0:i0 + len(ee)] = (dst[ee] - lo) % 128
        assert g1.min() >= 0 and g1.max() < REG
        # transposed one-hot: ohT[d, j] = 1 if dstv[j] == d (pad cols = 0)
        ohT = np.zeros((128, TOT), dtype=np.float16)
        real = dstv < 128
        ohT[dstv[real].astype(np.int64), np.nonzero(real)[0]] = 1.0
        s3 = np.full(NSL, PAD3, dtype=np.int64)
        s3[:NLOC] = slots3
        wgp = np.full(NW, -1, dtype=np.int64)
        wgp[:len(wg)] = wg
        cmb = np.full((128, len(strides)), BIGNEG, dtype=np.float32)
        for k, s in enumerate(strides):
            for i in range(NW - s):
                if wgp[i] >= 0 and wgp[i] == wgp[i + s]:
                    cmb[i, k] = 0.0
        wplace = np.full(128, G, dtype=np.int64)
        for i in range(len(wg)):
            if fwin[i]:
                wplace[i] = wg[i]
        onehot = np.zeros((NTL, 128, G), dtype=np.float32)
        nn = np.arange(NLOC)
        onehot[nn // 128, nn % 128, batch[lo:lo + NLOC]] = 1.0
        data.append(dict(
            g1=_wrap_idx(g1),
            dstv=_to_bf16(np.ascontiguousarray(
                dstv.reshape(TOT // 128, 128).T)),
            ohT=ohT,
            s3=_wrap_idx(s3),
            cmb=cmb,
            wplace=_wrap_idx(wplace),
            onehot=_to_bf16(onehot),
        ))
    inv_cnt = np.tile((1.0 / np.maximum(counts_g, 1.0))
                      .astype(np.float32)[None, :], (96, 1))
    return data, inv_cnt


def _to_bf16(a):
    return np.ascontiguousarray(a, dtype=np.float32).astype(np.float16)


def prep_float_inputs(cfg, inputs):
    NC, NLOC, NSL, FEAT = cfg["NC"], cfg["NLOC"], cfg["NSL"], cfg["FEAT"]
    f = {}
    for l in (1, 2, 3):
        W = np.asarray(inputs[f"W{l}"], np.float32)
        A = np.zeros((HC, 2 * H), np.float32)
        for h in range(H):
            A[h * C:(h + 1) * C, h] = np.asarray(inputs[f"a_src{l}"], np.float32)[h]
            A[h * C:(h + 1) * C, H + h] = np.asarray(inputs[f"a_dst{l}"], np.float32)[h]
        f[f"Waug{l}"] = _to_bf16(np.concatenate([W, W @ A], axis=1))
        f[f"brep{l}"] = _to_bf16(np.tile(
            np.asarray(inputs[f"b{l}"], np.float32)[None, :], (128, 1)))
    f["fc1_w"] = np.asarray(inputs["fc1_w"], np.float32)
    f["fc1_b"] = np.asarray(inputs["fc1_b"], np.float32).reshape(-1, 1)
    f["out_w"] = np.asarray(inputs["out_w"], np.float32)
    f["out_b"] = np.asarray(inputs["out_b"], np.float32).reshape(-1, 1)
    return f


def build_in_maps(cfg, inputs, data, inv_cnt, fl):
    NC, NLOC, NSL = cfg["NC"], cfg["NLOC"], cfg["NSL"]
    in_maps = []
    for c in range(NC):
        m = dict(fl)
        m["inv_cnt"] = inv_cnt
        fto = np.zeros((cfg["FEAT"], NSL), np.float32)
        fto[:, :NLOC] = np.asarray(inputs["features"],
                                   np.float32)[c * NLOC:(c + 1) * NLOC].T
        m["featTown"] = _to_bf16(fto)
        m.update({k: data[c][k] for k in
                  ("g1", "dstv", "ohT", "s3", "cmb", "wplace", "onehot")})
        in_maps.append(m)
    return in_maps


def postprocess_output(cfg, y):
    return y.T.copy()


def build_program(cfg, mock_coll=False, phases="all"):
    """phases: all | nopool | noagg | gath | dense — deletion levels for
    HW time attribution (outputs invalid except 'all')."""
    from concourse import bacc, bass, mybir, tile
    from concourse.masks import make_identity
    f32, bf16, i16 = mybir.dt.float32, mybir.dt.float16, mybir.dt.int16
    AF, ALU = mybir.ActivationFunctionType, mybir.AluOpType
    G, NC, FEAT = cfg["G"], cfg["NC"], cfg["FEAT"]
    NLOC, NSL, GSLOTS, REG = cfg["NLOC"], cfg["NSL"], cfg["GSLOTS"], cfg["REG"]
    TOT, PAD3 = cfg["TOT"], cfg["PAD3"]
    NW, NT3, WIN, NTL = cfg["NW"], cfg["NT3"], cfg["WIN"], cfg["NTL"]
    NPW = 128 // WIN
    bsz, offs, nreg = cfg["bsz"], cfg["offs"], cfg["nreg"]
    SMAX, SMAXT = cfg["SMAX"], cfg["SMAXT"]
    core_ids = list(range(NC))

    import os
    # 3072-desc SWDGE ring: one gather's descriptors fit without forcing
    # desc-gen to stall on transfer drain (measured 1.3x speedup vs 1024)
    ddss = int(os.environ.get("DDSS", "49152"))
    nc = bacc.Bacc(None, num_devices=NC, num_swdge_queues=2,
                   dynamic_dma_scratch_size=ddss)

    featTo = nc.declare_dram_parameter("featTown", [FEAT, NSL], bf16, False)
    Waug, brep = [], []
    for l in (1, 2, 3):
        Waug.append(nc.declare_dram_parameter(
            f"Waug{l}", [FEAT if l == 1 else HC, HC + 2 * H], bf16, False))
        brep.append(nc.declare_dram_parameter(f"brep{l}", [128, HC], bf16, False))
    fc1_w = nc.declare_dram_parameter("fc1_w", [2 * HC, 48], f32, False)
    fc1_b = nc.declare_dram_parameter("fc1_b", [48, 1], f32, False)
    out_w = nc.declare_dram_parameter("out_w", [48, 2], f32, False)
    out_b = nc.declare_dram_parameter("out_b", [2, 1], f32, False)
    inv_cnt = nc.declare_dram_parameter("inv_cnt", [96, G], f32, False)
    g1i = nc.declare_dram_parameter("g1", [128, TOT // 16], i16, False)
    f8 = mybir.dt.float8e4
    dstvi = nc.declare_dram_parameter("dstv", [128, TOT // 128], bf16, False)
    ohTi = nc.declare_dram_parameter("ohT", [128, TOT], bf16, False)
    s3i = nc.declare_dram_parameter("s3", [128, NSL // 16], i16, False)
    cmbi = nc.declare_dram_parameter("cmb", [128, 6], f32, False)
    wplacei = nc.declare_dram_parameter("wplace", [128, 8], i16, False)
    onehoti = nc.declare_dram_parameter("onehot", [NTL, 128, G], bf16, False)
    yout = nc.declare_dram_parameter("y", [2, G], f32, True)

    htabOwn = nc.dram_tensor("htabOwn", [NSL, ROWB], bf16)
    htab = nc.dram_tensor("htab", [NC, NSL, ROWB], bf16, addr_space="Shared")
    padgrid = nc.dram_tensor("padgrid", [PAD3 + 128, HC], f32)
    maxgrid = nc.dram_tensor("maxgrid", [G + 1, HC], f32)
    poolsl = nc.dram_tensor("poolsl", [96, 4, G], f32)
    poolag = nc.dram_tensor("poolag", [NC, 96, 4, G], f32, addr_space="Shared")
    htabf = htab[:].rearrange("c n k -> (c n) k")

    with tile.TileContext(nc) as tc:
        with (
            tc.tile_pool(name="const", bufs=1) as constp,
            tc.tile_pool(name="wpool", bufs=1) as wpool,
            tc.tile_pool(name="dense", bufs=2) as densep,
            tc.tile_pool(name="edge", bufs=2) as edgep,
            tc.tile_pool(name="post", bufs=2) as postp,
            tc.tile_pool(name="xt", bufs=1) as xtp,
            tc.tile_pool(name="psum", bufs=2, space="PSUM") as psump,
            tc.tile_pool(name="psumA", bufs=2, space="PSUM") as psumAp,
            tc.tile_pool(name="psumP", bufs=1, space="PSUM") as psumPp,
            tc.tile_pool(name="small", bufs=2) as smallp,
        ):
            ident = constp.tile([128, 128], bf16)
            make_identity(nc, ident[:])
            identf = constp.tile([128, 128], f32)
            make_identity(nc, identf[:])
            # iota_rep[p, q*128 + d] = d  (bf16), q in [0, SMAX)
            iotai = constp.tile([128, 128], mybir.dt.int32)
            nc.gpsimd.iota(iotai[:], pattern=[[1, 128]], base=0,
                           channel_multiplier=0)
            iota1 = constp.tile([128, 128], bf16)
            nc.vector.tensor_copy(iota1[:], iotai[:])
            iota_rep = constp.tile([128, SMAXT, 128], bf16)
            for q in range(SMAXT):
                nc.sync.dma_start(iota_rep[:, q], iota1[:])
            m4 = constp.tile([128, 1], f32, tag="m4")
            nc.vector.memset(m4[:], -4.0)

            wtA, wtB, bt = [], [], []
            for l in range(3):
                ka = FEAT if l == 0 else 96
                a = wpool.tile([ka, HC + 2 * H], bf16, tag=f"wtA{l}")
                nc.sync.dma_start(a[:], Waug[l][:ka])
                wtA.append(a)
                if l == 0:
                    wtB.append(None)
                else:
                    b_ = wpool.tile([96, HC + 2 * H], bf16, tag=f"wtB{l}")
                    nc.sync.dma_start(b_[:], Waug[l][96:])
                    wtB.append(b_)
                bb = wpool.tile([128, HC], bf16, tag=f"bt{l}")
                nc.sync.dma_start(bb[:], brep[l][:])
                bt.append(bb)
            g1t = wpool.tile([128, TOT // 16], i16, tag="ixg1")
            nc.sync.dma_start(g1t[:], g1i[:])
            s3t = wpool.tile([128, NSL // 16], i16, tag="ixs3")
            nc.sync.dma_start(s3t[:], s3i[:])
            dstvt = wpool.tile([128, TOT // 128], bf16, tag="dstv")
            nc.sync.dma_start(dstvt[:], dstvi[:])
            cmbt = wpool.tile([128, 6], f32, tag="cmb")
            nc.sync.dma_start(cmbt[:], cmbi[:])
            wplt = wpool.tile([128, 8], i16, tag="wpl")
            nc.sync.dma_start(wplt[:], wplacei[:])
            invt = wpool.tile([96, G], f32, tag="inv")
            nc.sync.dma_start(invt[:], inv_cnt[:])
            fc1wt = []
            for k in range(4):
                t = wpool.tile([96, 48], f32, tag=f"fc1{k}")
                nc.sync.dma_start(t[:], fc1_w[k * 96:(k + 1) * 96])
                fc1wt.append(t)
            fc1bt = wpool.tile([48, 1], f32, tag="fc1b")
            nc.sync.dma_start(fc1bt[:], fc1_b[:])
            outwt = wpool.tile([48, 2], f32, tag="outw")
            nc.sync.dma_start(outwt[:], out_w[:])
            outbt = wpool.tile([2, 1], f32, tag="outb")
            nc.sync.dma_start(outbt[:], out_b[:])

            xTown = xtp.tile([96, 2, NSL], bf16, tag="xTown")
            if phases in ("noagg", "gath", "dense"):
                nc.vector.memset(xTown[:], 0.0)
            # s rows of OWN nodes, per dense tile: [s_src(4) | s_dst(4)]
            sown = xtp.tile([128, NTL, 2 * H], bf16, tag="sown")

            # zero padgrid (pooling scratch)
            zt = constp.tile([128, 16, HC], f32, tag="zt")
            nc.vector.memset(zt[:], 0.0)
            r0 = 0
            while r0 < PAD3 + 128:
                rr = min(2048, PAD3 + 128 - r0)
                nc.sync.dma_start(
                    padgrid[r0:r0 + rr].rearrange("(p a) c -> p (a c)", p=128),
                    zt[:, :rr // 128].rearrange("p a c -> p (a c)"))
                r0 += rr

            WGRP = 8  # dense tiles per htabOwn write

            def dense_phase(l):
                """Own-node htab rows + s table; AllGather full htab."""
                for t0 in range(0, NTL, WGRP):
                    ng = min(WGRP, NTL - t0)
                    stage = densep.tile([128, WGRP, ROWB], bf16, tag="dstage")
                    nc.vector.memset(stage[:, :, HC + 2 * H:], 0.0)
                    for k in range(ng):
                        t = t0 + k
                        ps = psump.tile([128, HC + 2 * H], f32, tag="dps")
                        if l == 0:
                            ft = densep.tile([FEAT, 128], bf16, tag="fTo")
                            nc.sync.dma_start(
                                ft[:], featTo[:, t * 128:(t + 1) * 128])
                            nc.tensor.matmul(
                                ps[:], ft[:],
                                wtA[0][:], start=True, stop=True)
                        else:
                            nc.tensor.matmul(
                                ps[:], xTown[:, 0, t * 128:(t + 1) * 128],
                                wtA[l][:], start=True, stop=False)
                            nc.tensor.matmul(
                                ps[:], xTown[:, 1, t * 128:(t + 1) * 128],
                                wtB[l][:], start=False, stop=True)
                        nc.scalar.activation(stage[:, k, :HC + 2 * H], ps[:],
                                             AF.Copy)
                        nc.vector.tensor_copy(sown[:, t], ps[:, HC:])
                    nc.sync.dma_start(
                        htabOwn[t0 * 128:(t0 + ng) * 128].rearrange(
                            "(a p) k -> p a k", p=128),
                        stage[:, :ng])
                if mock_coll:
                    nc.sync.dma_start(htab[0], htabOwn[:])
                else:
                    nc.gpsimd.collective_compute(
                        "AllGather", mybir.AluOpType.bypass,
                        replica_groups=[core_ids],
                        ins=[htabOwn[:]], outs=[htab[:]])

            def agg_tile(l, dt):
                """Aggregate messages for dst-tile dt into psum [128, 196].

                Both src-regions of the tile are gathered into ONE fused
                block (contiguous in edge order) so the one-hot build, the
                s_dst broadcast, and the w/sc pipeline each run once per
                tile."""
                i0 = int(offs[dt, 0])
                assert int(offs[dt, 1]) == i0 + int(bsz[dt, 0])
                ST = int(bsz[dt, 0] + bsz[dt, 1])
                ncT = ST // 128
                big = edgep.tile([128, SMAXT, ROWB], bf16, tag="big")
                c0 = 0
                for r in range(nreg):
                    S = int(bsz[dt, r])
                    if S == 0:
                        continue
                    ir = int(offs[dt, r])
                    nc.gpsimd.dma_gather(
                        big[:, c0:c0 + S // 128],
                        htabf[r * REG:(r + 1) * REG],
                        g1t[:, ir // 16:(ir + S) // 16], S, S, ROWB,
                        single_packet=False, queue_num=r % 2)
                    c0 += S // 128
                # transposed one-hot for this tile (host-precomputed, fp8)
                ohT = edgep.tile([128, SMAXT * 128], bf16, tag="ohT")
                nc.sync.dma_start(ohT[:, :ST], ohTi[:, i0:i0 + ST])
                if phases == "gath":
                    return None
                # forward one-hot: oh[p, q, d] = (dstv[p, q] == d)
                oh = edgep.tile([128, SMAXT, 128], bf16, tag="oh")
                nc.vector.tensor_tensor(
                    oh[:, :ncT], iota_rep[:, :ncT],
                    dstvt[:, i0 // 128:i0 // 128 + ncT]
                    .rearrange("p (q o) -> p q o", o=1)
                    .to_broadcast([128, ncT, 128]),
                    ALU.is_equal)
                # per-edge s_dst via TensorE: ohT_col^T @ sdst_tile
                pss = psump.tile([128, SMAXT * H], f32, tag="dps")
                for q in range(ncT):
                    nc.tensor.matmul(
                        pss[:, q * H:(q + 1) * H],
                        ohT[:, q * 128:(q + 1) * 128],
                        sown[:, dt, H:2 * H],
                        start=True, stop=True)
                w = edgep.tile([128, SMAXT, H], f32, tag="w")
                nc.vector.tensor_add(
                    w[:, :ncT],
                    pss[:, :ncT * H].rearrange("p (q h) -> p q h", h=H),
                    big[:, :ncT, HC:HC + H])
                wl = edgep.tile([128, SMAXT, H], f32, tag="wl")
                nc.scalar.activation(wl[:, :ncT], w[:, :ncT],
                                     AF.Copy, scale=NEG)
                nc.vector.tensor_max(w[:, :ncT], w[:, :ncT], wl[:, :ncT])
                nc.scalar.activation(w[:, :ncT], w[:, :ncT], AF.Exp,
                                     bias=m4[:])
                sc = edgep.tile([128, SMAXT, HC + H], bf16, tag="sc")
                nc.vector.tensor_mul(
                    sc[:, :ncT, :HC].rearrange("p q (h c) -> p q h c", h=H),
                    big[:, :ncT, :HC].rearrange("p q (h c) -> p q h c", h=H),
                    w[:, :ncT].rearrange("p q (h o) -> p q h o", o=1)
                    .to_broadcast([128, ncT, H, C]))
                nc.scalar.activation(sc[:, :ncT, HC:], w[:, :ncT], AF.Copy)
                if phases == "noagg":
                    return None
                ps = psumAp.tile([128, HC + H], f32, tag="agg")
                for q in range(ncT):
                    nc.tensor.matmul(ps[:], oh[:, q], sc[:, q],
                                     start=(q == 0), stop=(q == ncT - 1))
                return ps

            def post_tile(l, ps):
                """1/den, +bias, ELU -> y f32 [128, HC] (+ bf16 copy)."""
                den = postp.tile([128, H], f32, tag="pden")
                nc.vector.tensor_scalar(den[:], ps[:, HC:], 1e-16, None,
                                        ALU.max)
                nc.vector.reciprocal(den[:], den[:])
                y = postp.tile([128, HC], f32, tag="py")
                nc.vector.tensor_mul(
                    y[:].rearrange("p (h c) -> p h c", h=H),
                    ps[:, :HC].rearrange("p (h c) -> p h c", h=H),
                    den[:].rearrange("p (h o) -> p h o", o=1)
                    .to_broadcast([128, H, C]))
                nc.vector.tensor_add(y[:], y[:], bt[l][:])
                e = postp.tile([128, HC], f32, tag="pe")
                nc.vector.tensor_scalar(e[:], y[:], 0.0, None, ALU.min)
                nc.scalar.activation(e[:], e[:], AF.Exp)
                nc.vector.tensor_scalar(e[:], e[:], -1.0, None, ALU.add)
                nc.vector.tensor_max(y[:], y[:], e[:])
                y16 = postp.tile([128, HC], bf16, tag="py16")
                nc.scalar.activation(y16[:], y[:], AF.Copy)
                return y, y16

            for l in range(3):
                dense_phase(l)
                if phases == "dense":
                    continue
                if phases in ("gath", "noagg"):
                    for t in range(NTL):
                        agg_tile(l, t)
                    continue
                if l < 2:
                    for t in range(NTL):
                        ps = agg_tile(l, t)
                        y, y16 = post_tile(l, ps)
                        for blk in range(2):
                            pt = psump.tile([96, 128], bf16, tag="tps")
                            nc.tensor.transpose(
                                pt[:], y16[:, blk * 96:(blk + 1) * 96],
                                ident[:])
                            nc.scalar.activation(
                                xTown[:, blk, t * 128:(t + 1) * 128], pt[:],
                                AF.Copy)
                elif phases == "nopool":
                    for t in range(NTL):
                        ps = agg_tile(l, t)
                        post_tile(l, ps)
                else:
                    sump0 = psumPp.tile([96, G], f32, tag="sum0")
                    sump1 = psumPp.tile([96, G], f32, tag="sum1")
                    sump = [sump0, sump1]
                    for t in range(NTL):
                        ps = agg_tile(l, t)
                        y, y16 = post_tile(l, ps)
                        oht = postp.tile([128, G], bf16, tag="oht")
                        nc.sync.dma_start(oht[:], onehoti[t])
                        for blk in range(2):
                            nc.tensor.matmul(
                                sump[blk][:], y16[:, blk * 96:(blk + 1) * 96],
                                oht[:], start=(t == 0),
                                stop=(t == NTL - 1))
                        yo = postp.tile([128, HC], f32, tag="pyo")
                        nc.vector.tensor_scalar(yo[:], y[:], MAXOFF, None,
                                                ALU.add)
                        nc.gpsimd.dma_scatter_add(
                            padgrid[:, :],
                            yo[:].rearrange("p (a c) -> p a c", a=1),
                            s3t[:, t * 8:(t + 1) * 8], 128, 128, HC,
                            single_packet=False)
                    wmax = smallp.tile([96, 2, NW], f32, tag="wmax")
                    for t in range(NT3):
                        rows = postp.tile([128, HC], f32, tag="prow3")
                        nc.sync.dma_start(rows[:],
                                          padgrid[t * 128:(t + 1) * 128])
                        for blk in range(2):
                            pt = psump.tile([96, 128], f32, tag="tps")
                            nc.tensor.transpose(
                                pt[:], rows[:, blk * 96:(blk + 1) * 96],
                                identf[:])
                            nc.vector.tensor_reduce(
                                wmax[:, blk, t * NPW:(t + 1) * NPW],
                                pt[:].rearrange("p (w q) -> p w q", q=WIN),
                                mybir.AxisListType.X, ALU.max)
                    wrow = smallp.tile([128, HC], f32, tag="wrow")
                    nc.vector.memset(wrow[:], 0.0)
                    for blk in range(2):
                        pt2 = psump.tile([128, 96], f32, tag="tps")
                        nc.tensor.transpose(pt2[:NW], wmax[:, blk],
                                            identf[:96, :96])
                        nc.vector.tensor_copy(
                            wrow[:NW, blk * 96:(blk + 1) * 96], pt2[:NW])
                    for ki, s in enumerate([1, 2, 4, 8, 16, 32]):
                        if s >= NW:
                            break
                        sh = smallp.tile([128, HC], f32, tag="wsh")
                        nc.sync.dma_start(sh[:NW - s], wrow[s:NW])
                        nc.vector.tensor_scalar(sh[:NW - s], sh[:NW - s],
                                                cmbt[:NW - s, ki:ki + 1],
                                                None, ALU.add)
                        nc.vector.tensor_max(wrow[:NW - s], wrow[:NW - s],
                                             sh[:NW - s])
                    zg = smallp.tile([G + 1, HC], f32, tag="zg")
                    nc.vector.memset(zg[:], 0.0)
                    nc.sync.dma_start(maxgrid[:], zg[:])
                    nc.gpsimd.dma_scatter_add(
                        maxgrid[:], wrow[:].rearrange("p (a c) -> p a c", a=1),
                        wplt[:], 128, 128, HC, single_packet=False)
                    mg = smallp.tile([G, HC], f32, tag="mg")
                    nc.sync.dma_start(mg[:], maxgrid[:G])
                    pp = smallp.tile([96, 4, G], f32, tag="pp")
                    for blk in range(2):
                        nc.vector.tensor_copy(pp[:, blk], sump[blk][:])
                        pt3 = psump.tile([96, G], f32, tag="tps")
                        nc.tensor.transpose(
                            pt3[:], mg[:, blk * 96:(blk + 1) * 96],
                            identf[:G, :G])
                        nc.vector.tensor_copy(pp[:, 2 + blk], pt3[:])
                    nc.sync.dma_start(poolsl[:], pp[:])
                    if mock_coll:
                        nc.sync.dma_start(poolag[0], poolsl[:])
                    else:
                        nc.gpsimd.collective_compute(
                            "AllGather", mybir.AluOpType.bypass,
                            replica_groups=[core_ids],
                            ins=[poolsl[:]], outs=[poolag[:]])
                    agg = smallp.tile([96, 4, G], f32, tag="agg2")
                    for c_ in range(NC):
                        at = smallp.tile([96, 4, G], f32, tag="agt")
                        nc.sync.dma_start(at[:], poolag[c_])
                        if c_ == 0:
                            nc.vector.tensor_copy(agg[:], at[:])
                        else:
                            nc.vector.tensor_add(agg[:, :2], agg[:, :2],
                                                 at[:, :2])
                            nc.vector.tensor_max(agg[:, 2:], agg[:, 2:],
                                                 at[:, 2:])
                    for blk in range(2):
                        nc.vector.tensor_mul(agg[:, blk], agg[:, blk], invt[:])
                        nc.vector.tensor_scalar(agg[:, 2 + blk],
                                                agg[:, 2 + blk],
                                                -MAXOFF, None, ALU.add)
                    zp = psump.tile([48, G], f32, tag="tps")
                    for k in range(4):
                        nc.tensor.matmul(zp[:], fc1wt[k], agg[:, k],
                                         start=(k == 0), stop=(k == 3))
                    z = smallp.tile([48, G], f32, tag="z")
                    nc.vector.tensor_scalar(z[:], zp[:], fc1bt[:], None,
                                            ALU.add)
                    e2 = smallp.tile([48, G], f32, tag="e2")
                    nc.vector.tensor_scalar(e2[:], z[:], 0.0, None, ALU.min)
                    nc.scalar.activation(e2[:], e2[:], AF.Exp)
                    nc.vector.tensor_scalar(e2[:], e2[:], -1.0, None, ALU.add)
                    nc.vector.tensor_max(z[:], z[:], e2[:])
                    yp = psump.tile([2, G], f32, tag="tps")
                    nc.tensor.matmul(yp[:], outwt[:], z[:], start=True,
                                     stop=True)
                    yf = smallp.tile([2, G], f32, tag="yf")
                    nc.vector.tensor_scalar(yf[:], yp[:], outbt[:], None,
                                            ALU.add)
                    nc.sync.dma_start(yout[:], yf[:])
    nc.finalize()
    return nc


def run(inputs, cfg, **run_kw):
    data, inv_cnt = host_prep(cfg, inputs["adj"], inputs["batch"])
    fl = prep_float_inputs(cfg, inputs)
    NC = cfg["NC"]
    in_maps = build_in_maps(cfg, inputs, data, inv_cnt, fl)
    nc = build_program(cfg)
    from concourse.bass_utils import run_bass_kernel_spmd
    res = run_bass_kernel_spmd(nc, in_maps, list(range(NC)), **run_kw)
    y = np.asarray(res.results[0]["y"])
    return postprocess_output(cfg, y), res


def kernel(**inputs):
    y, _ = run(inputs, make_cfg())
    return y


# revision 63
# speedup vs baseline: 1.3698x; 1.3698x over previous
"""3-layer GAT + graph pooling + MLP on 8 Trainium2 NeuronCores (Bass).

fp16 edge pipeline, non-redundant dense, no per-edge s_dst gather.

Sharding: core c owns dst-nodes [c*NLOC, (c+1)*NLOC) and their in-edges.
Per layer:
  dense:  OWN nodes only: htab rows [h(192) | s_src(4) | s_dst(4) | pad]
          as 256-elem fp16 rows (512B, the dma_gather minimum granularity
          with latency-multiplier 1) via one augmented-W matmul per tile;
          one AllGather distributes the full table (links ~150GB/s).
  edge:   per dst-tile, BOTH src-regions gathered into one fused block
          (contiguous edge order) so every vector op runs once per tile:
          dma_gather 512B fp16 src rows (SWDGE ring enlarged to 3072
          descs so desc-gen never stalls on transfer drain; regions
          alternate the two SWDGE queues); per-edge s_dst broadcast on
          TensorE: lhsT=ohT (host-precomputed transposed one-hot, fp16)
          x rhs=own s_dst rows -> [128e, H] psum, no second gather;
          w = exp(lrelu(s_src + s_dst) - 4) in f32 (the -4 shift cancels
          in the softmax and keeps w in fp16-safe range downstream);
          sc = h * w_bcast in one DVE op; aggregation
          psum += oh^T @ [w*h | w] with oh built in ONE DVE is_equal per
          tile (iota_rep vs partition-major dstv, fp16 exact for 0..127).
  post:   1/den, +bias, ELU in f32 (max-pool path needs f32 accuracy),
          fp16 copy transposed to channel-major x_own (SBUF-resident)
          for the next layer's dense.
Regions: htab rows core-major; region r = rows [r*25088, (r+1)*25088)
so int16 gather indices fit. Sequential DMAs issue from the sync (SP)
engine queue (HWDGE) to keep the Pool engine free for gather desc-gen.
Pooling: unique-index scatter (+1000 offset) into graph-aligned slots,
one-hot sum-pool on TensorE, 64-wide window max + log-depth same-graph
combine, one small AllGather, MLP computed redundantly. dma_scatter_add
is collision-safe only with unique indices, which these are.
All data-dependent structure (indices, one-hots) is INPUT DATA; the
program is static and identical across cores (SPMD).
"""

import sys
import numpy as np

sys.path.insert(0, "/opt/trn_rl_repo")

H, C = 4, 48
HC = H * C          # 192
NEG = 0.2
BIGNEG = -2.0e30
MAXOFF = 1000.0     # max-pool offset: x3 = elu(...) >= -1, so x3+1000 > 0
ROWB = 256          # htab row elems (bf16): h 192 | s_src 4 | s_dst 4 | pad


def make_cfg(N=50000, E=800000, G=64, NC=8, FEAT=128, WIN=64):
    NLOC = N // NC
    assert NLOC * NC == N
    NSL = ((NLOC + 127) // 128) * 128
    GSLOTS = NC * NSL
    REG = GSLOTS // 2
    assert REG <= 32768
    return dict(N=N, E=E, G=G, NC=NC, FEAT=FEAT, NLOC=NLOC, NSL=NSL,
                GSLOTS=GSLOTS, REG=REG, WIN=WIN)


def _wrap_idx(idx):
    """SWDGE idx layout: element i -> [i % 16, i // 16], replicated to 128
    partitions (one copy per Q7 core)."""
    T = idx.shape[0]
    out = np.ascontiguousarray(idx.reshape(T // 16, 16).T).astype(np.int16)
    return np.tile(out, (8, 1))


def host_prep(cfg, adj, batch):
    N, G, NC = cfg["N"], cfg["G"], cfg["NC"]
    NLOC, NSL, WIN, REG = cfg["NLOC"], cfg["NSL"], cfg["WIN"], cfg["REG"]
    NTL = NSL // 128
    nreg = 2
    src = np.asarray(adj[0], dtype=np.int64)
    dst = np.asarray(adj[1], dtype=np.int64)
    batch = np.asarray(batch, dtype=np.int64)
    src_slot = (src // NLOC) * NSL + (src % NLOC)
    counts_g = np.bincount(batch, minlength=G)

    # ---- edge grouping: (dst-tile, src-region) blocks, each padded %128
    blocks_all = []
    sizes = np.zeros((NC, NTL, nreg), dtype=np.int64)
    for c in range(NC):
        lo = c * NLOC
        esel = np.nonzero((dst >= lo) & (dst < lo + NLOC))[0]
        # sort by src for gather locality
        esel = esel[np.argsort(src_slot[esel], kind="stable")]
        dt_of = (dst[esel] - lo) // 128
        rg_of = src_slot[esel] // REG
        d = {}
        for t in range(NTL):
            for r in range(nreg):
                ee = esel[(dt_of == t) & (rg_of == r)]
                d[(t, r)] = ee
                sizes[c, t, r] = len(ee)
        blocks_all.append(d)
    bsz = np.zeros((NTL, nreg), dtype=np.int64)
    for t in range(NTL):
        for r in range(nreg):
            m = int(sizes[:, t, r].max())
            if r == 0:
                m = max(m, 1)          # ensure >=1 tile so psum gets reset
            bsz[t, r] = -(-m // 128) * 128 if m else 0
    offs = np.zeros((NTL, nreg), dtype=np.int64)
    o = 0
    for t in range(NTL):
        for r in range(nreg):
            offs[t, r] = o
            o += bsz[t, r]
    TOT = int(o)

    # ---- layer-3 graph-aligned slots (for pooling) — as v1
    pad3_meta, pad3_tot = [], 0
    for c in range(NC):
        lo = c * NLOC
        b = batch[lo:lo + NLOC]
        gids, starts = np.unique(b, return_index=True)
        osort = np.argsort(starts)
        gids, starts = gids[osort], starts[osort]
        ends = np.append(starts[1:], NLOC)
        slots = np.empty(NLOC, dtype=np.int64)
        wg, fwin = [], []
        pos = 0
        for g, s, e in zip(gids, starts, ends):
            cnt = e - s
            slots[s:e] = pos + np.arange(cnt)
            nw = -(-cnt // WIN)
            wg += [int(g)] * nw
            fwin += [1] + [0] * (nw - 1)
            pos += nw * WIN
        pad3_meta.append((slots, wg, fwin))
        pad3_tot = max(pad3_tot, pos)
    PAD3 = -(-pad3_tot // 128) * 128
    NW, NT3 = PAD3 // WIN, PAD3 // 128
    assert NW <= 128
    SMAX = int(bsz.max()) // 128
    SMAXT = int(bsz.sum(axis=1).max()) // 128
    cfg.update(TOT=TOT, bsz=bsz, offs=offs, PAD3=PAD3, NW=NW, NT3=NT3,
               nreg=nreg, NTL=NTL, SMAX=SMAX, SMAXT=SMAXT)

    data = []
    strides = [1, 2, 4, 8, 16, 32]
    for c in range(NC):
        lo = c * NLOC
        slots3, wg, fwin = pad3_meta[c]
        g1 = np.zeros(TOT, dtype=np.int64)
        dstv = np.full(TOT, 999.0, dtype=np.float32)
        for t in range(NTL):
            for r in range(nreg):
                ee = blocks_all[c][(t, r)]
                i0 = int(offs[t, r])
                g1[i0:i0 + len(ee)] = src_slot[ee] - r * REG
                dstv[i0:i0 + len(ee)] = (dst[ee] - lo) % 128
        assert g1.min() >= 0 and g1.max() < REG
        # transposed one-hot: ohT[d, j] = 1 if dstv[j] == d (pad cols = 0)
        ohT = np.zeros((128, TOT), dtype=np.float16)
        real = dstv < 128
        ohT[dstv[real].astype(np.int64), np.nonzero(real)[0]] = 1.0
        s3 = np.full(NSL, PAD3, dtype=np.int64)
        s3[:NLOC] = slots3
        wgp = np.full(NW, -1, dtype=np.int64)
        wgp[:len(wg)] = wg
        cmb = np.full((128, len(strides)), BIGNEG, dtype=np.float32)
        for k, s in enumerate(strides):
            for i in range(NW - s):
                if wgp[i] >= 0 and wgp[i] == wgp[i + s]:
                    cmb[i, k] = 0.0
        wplace = np.full(128, G, dtype=np.int64)
        for i in range(len(wg)):
            if fwin[i]:
                wplace[i] = wg[i]
        onehot = np.zeros((NTL, 128, G), dtype=np.float32)
        nn = np.arange(NLOC)
        onehot[nn // 128, nn % 128, batch[lo:lo + NLOC]] = 1.0
        data.append(dict(
            g1=_wrap_idx(g1),
            dstv=_to_bf16(np.ascontiguousarray(
                dstv.reshape(TOT // 128, 128).T)),
            ohT=ohT,
            s3=_wrap_idx(s3),
            cmb=cmb,
            wplace=_wrap_idx(wplace),
            onehot=_to_bf16(onehot),
        ))
    inv_cnt = np.tile((1.0 / np.maximum(counts_g, 1.0))
                      .astype(np.float32)[None, :], (96, 1))
    return data, inv_cnt


def _to_bf16(a):
    return np.ascontiguousarray(a, dtype=np.float32).astype(np.float16)


def prep_float_inputs(cfg, inputs):
    NC, NLOC, NSL, FEAT = cfg["NC"], cfg["NLOC"], cfg["NSL"], cfg["FEAT"]
    f = {}
    for l in (1, 2, 3):
        W = np.asarray(inputs[f"W{l}"], np.float32)
        A = np.zeros((HC, 2 * H), np.float32)
        for h in range(H):
            A[h * C:(h + 1) * C, h] = np.asarray(inputs[f"a_src{l}"], np.float32)[h]
            A[h * C:(h + 1) * C, H + h] = np.asarray(inputs[f"a_dst{l}"], np.float32)[h]
        f[f"Waug{l}"] = _to_bf16(np.concatenate([W, W @ A], axis=1))
        f[f"brep{l}"] = _to_bf16(np.tile(
            np.asarray(inputs[f"b{l}"], np.float32)[None, :], (128, 1)))
    f["fc1_w"] = np.asarray(inputs["fc1_w"], np.float32)
    f["fc1_b"] = np.asarray(inputs["fc1_b"], np.float32).reshape(-1, 1)
    f["out_w"] = np.asarray(inputs["out_w"], np.float32)
    f["out_b"] = np.asarray(inputs["out_b"], np.float32).reshape(-1, 1)
    return f


def build_in_maps(cfg, inputs, data, inv_cnt, fl):
    NC, NLOC, NSL = cfg["NC"], cfg["NLOC"], cfg["NSL"]
    in_maps = []
    for c in range(NC):
        m = dict(fl)
        m["inv_cnt"] = inv_cnt
        fto = np.zeros((cfg["FEAT"], NSL), np.float32)
        fto[:, :NLOC] = np.asarray(inputs["features"],
                                   np.float32)[c * NLOC:(c + 1) * NLOC].T
        m["featTown"] = _to_bf16(fto)
        m.update({k: data[c][k] for k in
                  ("g1", "dstv", "ohT", "s3", "cmb", "wplace", "onehot")})
        in_maps.append(m)
    return in_maps


def postprocess_output(cfg, y):
    return y.T.copy()


def build_program(cfg, mock_coll=False, phases="all"):
    """phases: all | nopool | noagg | gath | dense — deletion levels for
    HW time attribution (outputs invalid except 'all')."""
    from concourse import bacc, bass, mybir, tile
    from concourse.masks import make_identity
    f32, bf16, i16 = mybir.dt.float32, mybir.dt.float16, mybir.dt.int16
    AF, ALU = mybir.ActivationFunctionType, mybir.AluOpType
    G, NC, FEAT = cfg["G"], cfg["NC"], cfg["FEAT"]
    NLOC, NSL, GSLOTS, REG = cfg["NLOC"], cfg["NSL"], cfg["GSLOTS"], cfg["REG"]
    TOT, PAD3 = cfg["TOT"], cfg["PAD3"]
    NW, NT3, WIN, NTL = cfg["NW"], cfg["NT3"], cfg["WIN"], cfg["NTL"]
    NPW = 128 // WIN
    bsz, offs, nreg = cfg["bsz"], cfg["offs"], cfg["nreg"]
    SMAX, SMAXT = cfg["SMAX"], cfg["SMAXT"]
    core_ids = list(range(NC))

    import os
    # 3072-desc SWDGE ring: one gather's descriptors fit without forcing
    # desc-gen to stall on transfer drain (measured 1.3x speedup vs 1024)
    ddss = int(os.environ.get("DDSS", "57344"))
    nc = bacc.Bacc(None, num_devices=NC, num_swdge_queues=2,
                   dynamic_dma_scratch_size=ddss)

    featTo = nc.declare_dram_parameter("featTown", [FEAT, NSL], bf16, False)
    Waug, brep = [], []
    for l in (1, 2, 3):
        Waug.append(nc.declare_dram_parameter(
            f"Waug{l}", [FEAT if l == 1 else HC, HC + 2 * H], bf16, False))
        brep.append(nc.declare_dram_parameter(f"brep{l}", [128, HC], bf16, False))
    fc1_w = nc.declare_dram_parameter("fc1_w", [2 * HC, 48], f32, False)
    fc1_b = nc.declare_dram_parameter("fc1_b", [48, 1], f32, False)
    out_w = nc.declare_dram_parameter("out_w", [48, 2], f32, False)
    out_b = nc.declare_dram_parameter("out_b", [2, 1], f32, False)
    inv_cnt = nc.declare_dram_parameter("inv_cnt", [96, G], f32, False)
    g1i = nc.declare_dram_parameter("g1", [128, TOT // 16], i16, False)
    f8 = mybir.dt.float8e4
    dstvi = nc.declare_dram_parameter("dstv", [128, TOT // 128], bf16, False)
    ohTi = nc.declare_dram_parameter("ohT", [128, TOT], bf16, False)
    s3i = nc.declare_dram_parameter("s3", [128, NSL // 16], i16, False)
    cmbi = nc.declare_dram_parameter("cmb", [128, 6], f32, False)
    wplacei = nc.declare_dram_parameter("wplace", [128, 8], i16, False)
    onehoti = nc.declare_dram_parameter("onehot", [NTL, 128, G], bf16, False)
    yout = nc.declare_dram_parameter("y", [2, G], f32, True)

    htabOwn = nc.dram_tensor("htabOwn", [NSL, ROWB], bf16)
    htab = nc.dram_tensor("htab", [NC, NSL, ROWB], bf16, addr_space="Shared")
    padgrid = nc.dram_tensor("padgrid", [PAD3 + 128, HC], f32)
    maxgrid = nc.dram_tensor("maxgrid", [G + 1, HC], f32)
    poolsl = nc.dram_tensor("poolsl", [96, 4, G], f32)
    poolag = nc.dram_tensor("poolag", [NC, 96, 4, G], f32, addr_space="Shared")
    htabf = htab[:].rearrange("c n k -> (c n) k")

    with tile.TileContext(nc) as tc:
        with (
            tc.tile_pool(name="const", bufs=1) as constp,
            tc.tile_pool(name="wpool", bufs=1) as wpool,
            tc.tile_pool(name="dense", bufs=2) as densep,
            tc.tile_pool(name="edge", bufs=2) as edgep,
            tc.tile_pool(name="post", bufs=2) as postp,
            tc.tile_pool(name="xt", bufs=1) as xtp,
            tc.tile_pool(name="psum", bufs=2, space="PSUM") as psump,
            tc.tile_pool(name="psumA", bufs=2, space="PSUM") as psumAp,
            tc.tile_pool(name="psumP", bufs=1, space="PSUM") as psumPp,
            tc.tile_pool(name="small", bufs=2) as smallp,
        ):
            ident = constp.tile([128, 128], bf16)
            make_identity(nc, ident[:])
            identf = constp.tile([128, 128], f32)
            make_identity(nc, identf[:])
            # iota_rep[p, q*128 + d] = d  (bf16), q in [0, SMAX)
            iotai = constp.tile([128, 128], mybir.dt.int32)
            nc.gpsimd.iota(iotai[:], pattern=[[1, 128]], base=0,
                           channel_multiplier=0)
            iota1 = constp.tile([128, 128], bf16)
            nc.vector.tensor_copy(iota1[:], iotai[:])
            iota_rep = constp.tile([128, SMAXT, 128], bf16)
            for q in range(SMAXT):
                nc.sync.dma_start(iota_rep[:, q], iota1[:])
            m4 = constp.tile([128, 1], f32, tag="m4")
            nc.vector.memset(m4[:], -4.0)

            wtA, wtB, bt = [], [], []
            for l in range(3):
                ka = FEAT if l == 0 else 96
                a = wpool.tile([ka, HC + 2 * H], bf16, tag=f"wtA{l}")
                nc.sync.dma_start(a[:], Waug[l][:ka])
                wtA.append(a)
                if l == 0:
                    wtB.append(None)
                else:
                    b_ = wpool.tile([96, HC + 2 * H], bf16, tag=f"wtB{l}")
                    nc.sync.dma_start(b_[:], Waug[l][96:])
                    wtB.append(b_)
                bb = wpool.tile([128, HC], bf16, tag=f"bt{l}")
                nc.sync.dma_start(bb[:], brep[l][:])
                bt.append(bb)
            g1t = wpool.tile([128, TOT // 16], i16, tag="ixg1")
            nc.sync.dma_start(g1t[:], g1i[:])
            s3t = wpool.tile([128, NSL // 16], i16, tag="ixs3")
            nc.sync.dma_start(s3t[:], s3i[:])
            dstvt = wpool.tile([128, TOT // 128], bf16, tag="dstv")
            nc.sync.dma_start(dstvt[:], dstvi[:])
            cmbt = wpool.tile([128, 6], f32, tag="cmb")
            nc.sync.dma_start(cmbt[:], cmbi[:])
            wplt = wpool.tile([128, 8], i16, tag="wpl")
            nc.sync.dma_start(wplt[:], wplacei[:])
            invt = wpool.tile([96, G], f32, tag="inv")
            nc.sync.dma_start(invt[:], inv_cnt[:])
            fc1wt = []
            for k in range(4):
                t = wpool.tile([96, 48], f32, tag=f"fc1{k}")
                nc.sync.dma_start(t[:], fc1_w[k * 96:(k + 1) * 96])
                fc1wt.append(t)
            fc1bt = wpool.tile([48, 1], f32, tag="fc1b")
            nc.sync.dma_start(fc1bt[:], fc1_b[:])
            outwt = wpool.tile([48, 2], f32, tag="outw")
            nc.sync.dma_start(outwt[:], out_w[:])
            outbt = wpool.tile([2, 1], f32, tag="outb")
            nc.sync.dma_start(outbt[:], out_b[:])

            onehott = wpool.tile([128, NTL, G], bf16, tag="onehot")
            nc.sync.dma_start(
                onehott[:], onehoti[:].rearrange("t p g -> p t g"))
            xTown = xtp.tile([96, 2, NSL], bf16, tag="xTown")
            if phases in ("noagg", "gath", "dense"):
                nc.vector.memset(xTown[:], 0.0)
            # s rows of OWN nodes, per dense tile: [s_src(4) | s_dst(4)]
            featTowns = xtp.tile([FEAT, NSL], bf16, tag="fTo")
            nc.sync.dma_start(featTowns[:], featTo[:])
            sown = xtp.tile([128, NTL, 2 * H], bf16, tag="sown")

            # zero padgrid (pooling scratch)
            zt = constp.tile([128, 16, HC], f32, tag="zt")
            nc.vector.memset(zt[:], 0.0)
            r0 = 0
            while r0 < PAD3 + 128:
                rr = min(2048, PAD3 + 128 - r0)
                nc.sync.dma_start(
                    padgrid[r0:r0 + rr].rearrange("(p a) c -> p (a c)", p=128),
                    zt[:, :rr // 128].rearrange("p a c -> p (a c)"))
                r0 += rr

            WGRP = 8  # dense tiles per htabOwn write

            def dense_phase(l):
                """Own-node htab rows + s table; AllGather full htab."""
                for t0 in range(0, NTL, WGRP):
                    ng = min(WGRP, NTL - t0)
                    stage = densep.tile([128, WGRP, ROWB], bf16, tag="dstage")
                    nc.vector.memset(stage[:, :, HC + 2 * H:], 0.0)
                    for k in range(ng):
                        t = t0 + k
                        ps = psump.tile([128, HC + 2 * H], f32, tag="dps")
                        if l == 0:
                            nc.tensor.matmul(
                                ps[:], featTowns[:, t * 128:(t + 1) * 128],
                                wtA[0][:], start=True, stop=True)
                        else:
                            nc.tensor.matmul(
                                ps[:], xTown[:, 0, t * 128:(t + 1) * 128],
                                wtA[l][:], start=True, stop=False)
                            nc.tensor.matmul(
                                ps[:], xTown[:, 1, t * 128:(t + 1) * 128],
                                wtB[l][:], start=False, stop=True)
                        nc.scalar.activation(stage[:, k, :HC + 2 * H], ps[:],
                                             AF.Copy)
                        nc.vector.tensor_copy(sown[:, t], ps[:, HC:])
                    nc.sync.dma_start(
                        htabOwn[t0 * 128:(t0 + ng) * 128].rearrange(
                            "(a p) k -> p a k", p=128),
                        stage[:, :ng])
                if mock_coll:
                    nc.sync.dma_start(htab[0], htabOwn[:])
                else:
                    nc.gpsimd.collective_compute(
                        "AllGather", mybir.AluOpType.bypass,
                        replica_groups=[core_ids],
                        ins=[htabOwn[:]], outs=[htab[:]])

            def agg_tile(l, dt):
                """Aggregate messages for dst-tile dt into psum [128, 196].

                Both src-regions of the tile are gathered into ONE fused
                block (contiguous in edge order) so the one-hot build, the
                s_dst broadcast, and the w/sc pipeline each run once per
                tile."""
                i0 = int(offs[dt, 0])
                assert int(offs[dt, 1]) == i0 + int(bsz[dt, 0])
                ST = int(bsz[dt, 0] + bsz[dt, 1])
                ncT = ST // 128
                big = edgep.tile([128, SMAXT, ROWB], bf16, tag="big")
                c0 = 0
                for r in range(nreg):
                    S = int(bsz[dt, r])
                    if S == 0:
                        continue
                    ir = int(offs[dt, r])
                    nc.gpsimd.dma_gather(
                        big[:, c0:c0 + S // 128],
                        htabf[r * REG:(r + 1) * REG],
                        g1t[:, ir // 16:(ir + S) // 16], S, S, ROWB,
                        single_packet=False, queue_num=r % 2)
                    c0 += S // 128
                # transposed one-hot for this tile (host-precomputed, fp8)
                ohT = edgep.tile([128, SMAXT * 128], bf16, tag="ohT")
                nc.sync.dma_start(ohT[:, :ST], ohTi[:, i0:i0 + ST])
                if phases == "gath":
                    return None
                # forward one-hot: oh[p, q, d] = (dstv[p, q] == d)
                oh = edgep.tile([128, SMAXT, 128], bf16, tag="oh")
                nc.vector.tensor_tensor(
                    oh[:, :ncT], iota_rep[:, :ncT],
                    dstvt[:, i0 // 128:i0 // 128 + ncT]
                    .rearrange("p (q o) -> p q o", o=1)
                    .to_broadcast([128, ncT, 128]),
                    ALU.is_equal)
                # per-edge s_dst via TensorE: ohT_col^T @ sdst_tile
                pss = psump.tile([128, SMAXT * H], f32, tag="dps")
                for q in range(ncT):
                    nc.tensor.matmul(
                        pss[:, q * H:(q + 1) * H],
                        ohT[:, q * 128:(q + 1) * 128],
                        sown[:, dt, H:2 * H],
                        start=True, stop=True)
                w = edgep.tile([128, SMAXT, H], f32, tag="w")
                nc.vector.tensor_add(
                    w[:, :ncT],
                    pss[:, :ncT * H].rearrange("p (q h) -> p q h", h=H),
                    big[:, :ncT, HC:HC + H])
                wl = edgep.tile([128, SMAXT, H], f32, tag="wl")
                nc.scalar.activation(wl[:, :ncT], w[:, :ncT],
                                     AF.Copy, scale=NEG)
                nc.vector.tensor_max(w[:, :ncT], w[:, :ncT], wl[:, :ncT])
                nc.scalar.activation(w[:, :ncT], w[:, :ncT], AF.Exp,
                                     bias=m4[:])
                sc = edgep.tile([128, SMAXT, HC + H], bf16, tag="sc")
                nc.vector.tensor_mul(
                    sc[:, :ncT, :HC].rearrange("p q (h c) -> p q h c", h=H),
                    big[:, :ncT, :HC].rearrange("p q (h c) -> p q h c", h=H),
                    w[:, :ncT].rearrange("p q (h o) -> p q h o", o=1)
                    .to_broadcast([128, ncT, H, C]))
                nc.scalar.activation(sc[:, :ncT, HC:], w[:, :ncT], AF.Copy)
                if phases == "noagg":
                    return None
                ps = psumAp.tile([128, HC + H], f32, tag="agg")
                for q in range(ncT):
                    nc.tensor.matmul(ps[:], oh[:, q], sc[:, q],
                                     start=(q == 0), stop=(q == ncT - 1))
                return ps

            def post_tile(l, ps):
                """1/den, +bias, ELU -> y f32 [128, HC] (+ bf16 copy)."""
                den = postp.tile([128, H], f32, tag="pden")
                nc.vector.tensor_scalar(den[:], ps[:, HC:], 1e-16, None,
                                        ALU.max)
                nc.vector.reciprocal(den[:], den[:])
                y = postp.tile([128, HC], f32, tag="py")
                nc.vector.tensor_mul(
                    y[:].rearrange("p (h c) -> p h c", h=H),
                    ps[:, :HC].rearrange("p (h c) -> p h c", h=H),
                    den[:].rearrange("p (h o) -> p h o", o=1)
                    .to_broadcast([128, H, C]))
                nc.vector.tensor_add(y[:], y[:], bt[l][:])
                e = postp.tile([128, HC], f32, tag="pe")
                nc.vector.tensor_scalar(e[:], y[:], 0.0, None, ALU.min)
                nc.scalar.activation(e[:], e[:], AF.Exp)
                nc.vector.tensor_scalar(e[:], e[:], -1.0, None, ALU.add)
                nc.vector.tensor_max(y[:], y[:], e[:])
                y16 = postp.tile([128, HC], bf16, tag="py16")
                nc.scalar.activation(y16[:], y[:], AF.Copy)
                return y, y16

            for l in range(3):
                dense_phase(l)
                if phases == "dense":
                    continue
                if phases in ("gath", "noagg"):
                    for t in range(NTL):
                        agg_tile(l, t)
                    continue
                if l < 2:
                    for t in range(NTL):
                        ps = agg_tile(l, t)
                        y, y16 = post_tile(l, ps)
                        for blk in range(2):
                            pt = psump.tile([96, 128], bf16, tag="tps")
                            nc.tensor.transpose(
                                pt[:], y16[:, blk * 96:(blk + 1) * 96],
                                ident[:])
                            nc.scalar.activation(
                                xTown[:, blk, t * 128:(t + 1) * 128], pt[:],
                                AF.Copy)
                elif phases == "nopool":
                    for t in range(NTL):
                        ps = agg_tile(l, t)
                        post_tile(l, ps)
                else:
                    sump0 = psumPp.tile([96, G], f32, tag="sum0")
                    sump1 = psumPp.tile([96, G], f32, tag="sum1")
                    sump = [sump0, sump1]
                    for t in range(NTL):
                        ps = agg_tile(l, t)
                        y, y16 = post_tile(l, ps)
                        for blk in range(2):
                            nc.tensor.matmul(
                                sump[blk][:], y16[:, blk * 96:(blk + 1) * 96],
                                onehott[:, t], start=(t == 0),
                                stop=(t == NTL - 1))
                        yo = postp.tile([128, HC], f32, tag="pyo")
                        nc.vector.tensor_scalar(yo[:], y[:], MAXOFF, None,
                                                ALU.add)
                        nc.gpsimd.dma_scatter_add(
                            padgrid[:, :],
                            yo[:].rearrange("p (a c) -> p a c", a=1),
                            s3t[:, t * 8:(t + 1) * 8], 128, 128, HC,
                            single_packet=False)
                    wmax = smallp.tile([96, 2, NW], f32, tag="wmax")
                    for t in range(NT3):
                        rows = postp.tile([128, HC], f32, tag="prow3")
                        nc.sync.dma_start(rows[:],
                                          padgrid[t * 128:(t + 1) * 128])
                        for blk in range(2):
                            pt = psump.tile([96, 128], f32, tag="tps")
                            nc.tensor.transpose(
                                pt[:], rows[:, blk * 96:(blk + 1) * 96],
                                identf[:])
                            nc.vector.tensor_reduce(
                                wmax[:, blk, t * NPW:(t + 1) * NPW],
                                pt[:].rearrange("p (w q) -> p w q", q=WIN),
                                mybir.AxisListType.X, ALU.max)
                    wrow = smallp.tile([128, HC], f32, tag="wrow")
                    nc.vector.memset(wrow[:], 0.0)
                    for blk in range(2):
                        pt2 = psump.tile([128, 96], f32, tag="tps")
                        nc.tensor.transpose(pt2[:NW], wmax[:, blk],
                                            identf[:96, :96])
                        nc.vector.tensor_copy(
                            wrow[:NW, blk * 96:(blk + 1) * 96], pt2[:NW])
                    for ki, s in enumerate([1, 2, 4, 8, 16, 32]):
                        if s >= NW:
                            break
                        sh = smallp.tile([128, HC], f32, tag="wsh")
                        nc.sync.dma_start(sh[:NW - s], wrow[s:NW])
                        nc.vector.tensor_scalar(sh[:NW - s], sh[:NW - s],
                                                cmbt[:NW - s, ki:ki + 1],
                                                None, ALU.add)
                        nc.vector.tensor_max(wrow[:NW - s], wrow[:NW - s],
                                             sh[:NW - s])
                    zg = smallp.tile([G + 1, HC], f32, tag="zg")
                    nc.vector.memset(zg[:], 0.0)
                    nc.sync.dma_start(maxgrid[:], zg[:])
                    nc.gpsimd.dma_scatter_add(
                        maxgrid[:], wrow[:].rearrange("p (a c) -> p a c", a=1),
                        wplt[:], 128, 128, HC, single_packet=False)
                    mg = smallp.tile([G, HC], f32, tag="mg")
                    nc.sync.dma_start(mg[:], maxgrid[:G])
                    pp = smallp.tile([96, 4, G], f32, tag="pp")
                    for blk in range(2):
                        nc.vector.tensor_copy(pp[:, blk], sump[blk][:])
                        pt3 = psump.tile([96, G], f32, tag="tps")
                        nc.tensor.transpose(
                            pt3[:], mg[:, blk * 96:(blk + 1) * 96],
                            identf[:G, :G])
                        nc.vector.tensor_copy(pp[:, 2 + blk], pt3[:])
                    nc.sync.dma_start(poolsl[:], pp[:])
                    if mock_coll:
                        nc.sync.dma_start(poolag[0], poolsl[:])
                    else:
                        nc.gpsimd.collective_compute(
                            "AllGather", mybir.AluOpType.bypass,
                            replica_groups=[core_ids],
                            ins=[poolsl[:]], outs=[poolag[:]])
                    agg = smallp.tile([96, 4, G], f32, tag="agg2")
                    for c_ in range(NC):
                        at = smallp.tile([96, 4, G], f32, tag="agt")
                        nc.sync.dma_start(at[:], poolag[c_])
                        if c_ == 0:
                            nc.vector.tensor_copy(agg[:], at[:])
                        else:
                            nc.vector.tensor_add(agg[:, :2], agg[:, :2],
                                                 at[:, :2])
                            nc.vector.tensor_max(agg[:, 2:], agg[:, 2:],
                                                 at[:, 2:])
                    for blk in range(2):
                        nc.vector.tensor_mul(agg[:, blk], agg[:, blk], invt[:])
                        nc.vector.tensor_scalar(agg[:, 2 + blk],
                                                agg[:, 2 + blk],
                                                -MAXOFF, None, ALU.add)
                    zp = psump.tile([48, G], f32, tag="tps")
                    for k in range(4):
                        nc.tensor.matmul(zp[:], fc1wt[k], agg[:, k],
                                         start=(k == 0), stop=(k == 3))
                    z = smallp.tile([48, G], f32, tag="z")
                    nc.vector.tensor_scalar(z[:], zp[:], fc1bt[:], None,
                                            ALU.add)
                    e2 = smallp.tile([48, G], f32, tag="e2")
                    nc.vector.tensor_scalar(e2[:], z[:], 0.0, None, ALU.min)
                    nc.scalar.activation(e2[:], e2[:], AF.Exp)
                    nc.vector.tensor_scalar(e2[:], e2[:], -1.0, None, ALU.add)
                    nc.vector.tensor_max(z[:], z[:], e2[:])
                    yp = psump.tile([2, G], f32, tag="tps")
                    nc.tensor.matmul(yp[:], outwt[:], z[:], start=True,
                                     stop=True)
                    yf = smallp.tile([2, G], f32, tag="yf")
                    nc.vector.tensor_scalar(yf[:], yp[:], outbt[:], None,
                                            ALU.add)
                    nc.sync.dma_start(yout[:], yf[:])
    nc.finalize()
    return nc


def run(inputs, cfg, **run_kw):
    data, inv_cnt = host_prep(cfg, inputs["adj"], inputs["batch"])
    fl = prep_float_inputs(cfg, inputs)
    NC = cfg["NC"]
    in_maps = build_in_maps(cfg, inputs, data, inv_cnt, fl)
    nc = build_program(cfg)
    from concourse.bass_utils import run_bass_kernel_spmd
    res = run_bass_kernel_spmd(nc, in_maps, list(range(NC)), **run_kw)
    y = np.asarray(res.results[0]["y"])
    return postprocess_output(cfg, y), res


def kernel(**inputs):
    y, _ = run(inputs, make_cfg())
    return y
